# revision 1
# baseline (speedup 1.0000x reference)
"""Trainium2 Bass kernel for nn_Attention_43413529428606 (linear attention
with l2-normed q/k, interleaved RoPE, mask, per-head power scaling).

Sharding: the 16384 (batch*seq) rows are split across 8 NeuronCores, 2048
rows each; cores 0-3 take batch 0, cores 4-7 batch 1.  Each core computes
q/k/v projections for its rows (all 16 heads), applies l2norm+RoPE+mask,
accumulates the per-head k^T v state, AllReduces that state (512 KB) within
its batch group, then applies attention and the output projection for its
rows.  The data path is fp16 (fp32 PSUM accumulation); q/attn/out phases
are fused per 512-row supertile so nothing spills to DRAM.  The q-side
mask is applied host-side on the output rows.

Self-contained: hardcodes all shapes; no sibling imports.
"""

import sys

for _p in ("/opt/trn_rl_repo",):
    if _p not in sys.path:
        sys.path.append(_p)

from contextlib import ExitStack

import numpy as np

import concourse.bass as bass
import concourse.bacc as bacc
import concourse.tile as tile
from concourse import mybir
from concourse.bass_utils import run_bass_kernel_spmd

F32 = mybir.dt.float32
F16 = mybir.dt.float16

DIM = 1024
H = 16
HD = 64
B = 2
C = 8192
ROPE_THETA = 10000.0

N_CORES = 8
R = (B * C) // N_CORES  # 2048 rows per core
NC_T = R // 128  # 16 c-tiles of 128 (phase A)
NQ_T = R // 512  # 4 c-supertiles of 512 (fused q/attn/out phase)
ND = DIM // 128  # 8 d-chunks
NJ = DIM // 128  # 8 j-tiles
NPAIR = H // 2  # 8 head pairs

Copy = mybir.ActivationFunctionType.Copy
Square = mybir.ActivationFunctionType.Square
Ln = mybir.ActivationFunctionType.Ln
Exp = mybir.ActivationFunctionType.Exp
MUL = mybir.AluOpType.mult
ADD = mybir.AluOpType.add


def build_nc(sim_mode=False, phases="ABC", reps=1):
    nc = bacc.Bacc(
        "TRN2",
        target_bir_lowering=False,
        debug=False,
        num_devices=1 if sim_mode else N_CORES,
    )

    # ---- DRAM parameters (per-core shapes, fp16 data path) ----
    xT = nc.dram_tensor("xT", [DIM, R], F16, kind="ExternalInput").ap()
    WkT = nc.dram_tensor("WkT", [DIM, DIM], F16, kind="ExternalInput").ap()
    WvT = nc.dram_tensor("WvT", [DIM, DIM], F16, kind="ExternalInput").ap()
    WqT = nc.dram_tensor("WqT", [DIM, DIM], F16, kind="ExternalInput").ap()
    WoT = nc.dram_tensor("WoT", [DIM, DIM], F16, kind="ExternalInput").ap()
    cosC = nc.dram_tensor("cosC", [R, HD], F16, kind="ExternalInput").ap()
    sinC = nc.dram_tensor("sinC", [R, HD], F16, kind="ExternalInput").ap()
    cosF = nc.dram_tensor("cosF", [128, R], F16, kind="ExternalInput").ap()
    sinF = nc.dram_tensor("sinF", [128, R], F16, kind="ExternalInput").ap()
    maskC = nc.dram_tensor("maskC", [128, NC_T], F32, kind="ExternalInput").ap()
    ind16T = nc.dram_tensor("ind16T", [DIM, 16], F16, kind="ExternalInput").ap()
    ind16 = nc.dram_tensor("ind16", [16, DIM], F16, kind="ExternalInput").ap()
    Pmat = nc.dram_tensor("Pmat", [128, 128], F16, kind="ExternalInput").ap()

    kv_in_d = nc.dram_tensor("kv_in_d", [128, NPAIR * 128], F32)
    kv_out_d = nc.dram_tensor("kv_out_d", [128, NPAIR * 128], F32)

    out_d = nc.dram_tensor("out", [DIM, R], F32, kind="ExternalOutput").ap()

    def blkview(dram_ap, csl):
        return dram_ap.rearrange("(t p) c -> p t c", p=128)[:, :, csl]

    with tile.TileContext(nc) as tc:
        with ExitStack() as ctx:
            consts = ctx.enter_context(tc.tile_pool(name="consts", bufs=1))
            kvblk_pool = ctx.enter_context(tc.tile_pool(name="kvblk", bufs=1))

            cosC_t = consts.tile([128, NC_T * HD], F16, tag="cosC")
            sinC_t = consts.tile([128, NC_T * HD], F16, tag="sinC")
            nc.sync.dma_start(
                out=cosC_t[:].rearrange("p (t f) -> p t f", t=NC_T),
                in_=cosC[:].rearrange("(t p) f -> p t f", p=128),
            )
            nc.sync.dma_start(
                out=sinC_t[:].rearrange("p (t f) -> p t f", t=NC_T),
                in_=sinC[:].rearrange("(t p) f -> p t f", p=128),
            )
            maskC_t = consts.tile([128, NC_T], F32, tag="maskC")
            ind16T_t = consts.tile([128, NJ * 16], F16, tag="ind16T")
            ind16_t = consts.tile([16, DIM], F16, tag="ind16")
            P_t = consts.tile([128, 128], F16, tag="Pmat")
            nc.sync.dma_start(out=maskC_t[:], in_=maskC[:])
            nc.sync.dma_start(
                out=ind16T_t[:].rearrange("p (t f) -> p t f", t=NJ),
                in_=ind16T[:].rearrange("(t p) f -> p t f", p=128),
            )
            nc.sync.dma_start(out=ind16_t[:], in_=ind16[:])
            nc.sync.dma_start(out=P_t[:], in_=Pmat[:])

            for _rep in range(reps):
              with ExitStack() as ctxX:
                xpool = ctxX.enter_context(tc.tile_pool(name="xpool", bufs=1))
                xT_all = xpool.tile([128, ND * R], F16, tag="xT")
                for xc in range(4):
                    nc.sync.dma_start(
                        out=xT_all[:, xc * 2 * R : (xc + 1) * 2 * R].rearrange(
                            "p (t c) -> p t c", t=2
                        ),
                        in_=xT[xc * 256 : (xc + 1) * 256, :].rearrange(
                            "(t p) c -> p t c", p=128
                        ),
                    )

                def xsl(dc, csl):
                    lo = dc * R
                    return xT_all[:, lo + csl.start : lo + csl.stop]

                if "B" in phases and "C" in phases:
                    wBC = ctxX.enter_context(tc.tile_pool(name="wBC", bufs=1))
                    wq_all = wBC.tile([128, ND * DIM], F16, tag="wq")
                    nc.sync.dma_start(
                        out=wq_all[:].rearrange("p (t f) -> p t f", t=ND),
                        in_=WqT[:].rearrange("(t p) f -> p t f", p=128),
                    )
                    wo_all = wBC.tile([128, ND * DIM], F16, tag="wo")
                    nc.scalar.dma_start(
                        out=wo_all[:].rearrange("p (t f) -> p t f", t=ND),
                        in_=WoT[:].rearrange("(t p) f -> p t f", p=128),
                    )
                    cosF_t = wBC.tile([128, R], F16, tag="cosF")
                    sinF_t = wBC.tile([128, R], F16, tag="sinF")
                    nc.sync.dma_start(out=cosF_t[:], in_=cosF[:])
                    nc.scalar.dma_start(out=sinF_t[:], in_=sinF[:])

                # ========= Phase A: k/v proj + process + kv Grams ==========
                with ExitStack() as ctxA:
                  if "A" in phases:
                    wA = ctxA.enter_context(tc.tile_pool(name="wA", bufs=1))
                    psA = ctxA.enter_context(
                        tc.tile_pool(name="psA", bufs=3, space="PSUM")
                    )
                    pskv = ctxA.enter_context(
                        tc.tile_pool(name="pskv", bufs=1, space="PSUM")
                    )
                    sbA = ctxA.enter_context(tc.tile_pool(name="sbA", bufs=2))
                    sb1 = ctxA.enter_context(tc.tile_pool(name="sb1", bufs=2))
                    smA = ctxA.enter_context(tc.tile_pool(name="smA", bufs=2))

                    wk_all = wA.tile([128, ND * DIM], F16, tag="wk")
                    wv_all = wA.tile([128, ND * DIM], F16, tag="wv")
                    for wt, wsrc in ((wk_all, WkT), (wv_all, WvT)):
                        for xc in range(2):
                            nc.scalar.dma_start(
                                out=wt[
                                    :, xc * 4 * DIM : (xc + 1) * 4 * DIM
                                ].rearrange("p (t f) -> p t f", t=4),
                                in_=wsrc[xc * 512 : (xc + 1) * 512, :].rearrange(
                                    "(t p) f -> p t f", p=128
                                ),
                            )

                    kv_ps = pskv.tile([128, NPAIR * 128], F32, tag="kvps")
                    kv_pending = []

                    # On HW start=True zeroes the whole PSUM bank, so only
                    # the first pair written to each bank may carry it.
                    def _emit_kv(item):
                        ct_, khat_, v_ = item
                        for p in range(NPAIR):
                            ps_ = slice(p * 128, (p + 1) * 128)
                            nc.tensor.matmul(
                                kv_ps[:, ps_],
                                khat_[:, ps_],
                                v_[:, ps_],
                                start=(
                                    True
                                    if sim_mode
                                    else (ct_ == 0 and p % 4 == 0)
                                ),
                                stop=(
                                    True if sim_mode else (ct_ == NC_T - 1)
                                ),
                            )

                    for ct in range(NC_T):
                        cs = slice(ct * 128, (ct + 1) * 128)
                        k_ps = psA.tile([128, DIM], F32, tag="proj_ps")
                        v_ps = psA.tile([128, DIM], F32, tag="proj_ps")
                        for half in range(2):
                            js = slice(half * 512, (half + 1) * 512)
                            for dc in range(ND):
                                nc.tensor.matmul(
                                    k_ps[:, js],
                                    xsl(dc, cs),
                                    wk_all[
                                        :, dc * DIM + js.start : dc * DIM + js.stop
                                    ],
                                    start=(dc == 0),
                                    stop=(dc == ND - 1),
                                )
                            for dc in range(ND):
                                nc.tensor.matmul(
                                    v_ps[:, js],
                                    xsl(dc, cs),
                                    wv_all[
                                        :, dc * DIM + js.start : dc * DIM + js.stop
                                    ],
                                    start=(dc == 0),
                                    stop=(dc == ND - 1),
                                )

                        # v: evict with mask fold (per-partition scale), cast
                        v_sb = sbA.tile([128, DIM], F16, tag="v_sb")
                        nc.scalar.activation(
                            v_sb[:], v_ps[:], Copy, scale=maskC_t[:, ct : ct + 1]
                        )
                        # k: evict fast (cast fp16) to free the PSUM slot
                        k_sb = sbA.tile([128, DIM], F16, tag="k_sb")
                        nc.scalar.activation(k_sb[:], k_ps[:], Copy)
                        # squares for the l2 norm
                        sq = sbA.tile([128, DIM], F16, tag="sq")
                        nc.scalar.activation(sq[:], k_ps[:], Square)

                        cosb = (
                            cosC_t[:, ct * HD : (ct + 1) * HD]
                            .unsqueeze(1)
                            .broadcast_to([128, H, HD])
                        )
                        sinb4 = (
                            sinC_t[:, ct * HD : (ct + 1) * HD]
                            .rearrange("p (g two) -> p g two", two=2)
                            .unsqueeze(1)
                            .broadcast_to([128, H, HD // 2, 2])
                        )
                        k3 = k_sb[:].rearrange("p (h f) -> p h f", h=H)
                        k_sw = k_sb[:].rearrange(
                            "p (h g two) -> p h g two", h=H, two=2
                        )[:, :, :, ::-1]

                        m1 = sb1.tile([128, DIM], F16, tag="m1")
                        nc.vector.tensor_tensor(
                            m1[:].rearrange("p (h f) -> p h f", h=H), k3, cosb, MUL
                        )
                        red = smA.tile([128, H], F32, tag="red")
                        nc.vector.tensor_reduce(
                            red[:],
                            sq[:].rearrange("p (h f) -> p h f", h=H),
                            mybir.AxisListType.X,
                            ADD,
                        )
                        lnr = smA.tile([128, H], F32, tag="lnr")
                        nc.scalar.activation(lnr[:], red[:], Ln)
                        rs = smA.tile([128, H], F32, tag="rs")
                        nc.scalar.activation(rs[:], lnr[:], Exp, scale=-0.5)
                        rsm = smA.tile([128, H], F32, tag="rsm")
                        nc.vector.tensor_scalar_mul(
                            rsm[:], rs[:], maskC_t[:, ct : ct + 1]
                        )
                        m2 = sb1.tile([128, DIM], F16, tag="m2")
                        nc.vector.tensor_tensor(
                            m2[:].rearrange("p (h g two) -> p h g two", h=H, two=2),
                            k_sw,
                            sinb4,
                            MUL,
                        )
                        s = sb1.tile([128, DIM], F16, tag="s")
                        nc.vector.tensor_tensor(s[:], m1[:], m2[:], ADD)
                        khat = sbA.tile([128, DIM], F16, tag="khat")
                        rsb = rsm[:].unsqueeze(2).broadcast_to([128, H, HD])
                        nc.vector.tensor_tensor(
                            khat[:].rearrange("p (h f) -> p h f", h=H),
                            s[:].rearrange("p (h f) -> p h f", h=H),
                            rsb,
                            MUL,
                        )

                        # kv Grams are issued one iteration late (software
                        # pipelining) so PE never waits on the khat chain
                        kv_pending.append((ct, khat, v_sb))
                        if len(kv_pending) > 1:
                            _emit_kv(kv_pending.pop(0))

                    while kv_pending:
                        _emit_kv(kv_pending.pop(0))

                    # evict kv partials and run the collective
                    kv_sb = sbA.tile([128, NPAIR * 128], F32, tag="kv_sb")
                    nc.vector.tensor_copy(kv_sb[:], kv_ps[:])
                    nc.sync.dma_start(out=kv_in_d.ap(), in_=kv_sb[:])
                    if sim_mode:
                        # stand-in for the AllReduce so TimelineSim can run
                        nc.sync.dma_start(out=kv_out_d.ap(), in_=kv_in_d.ap())
                    else:
                        nc.gpsimd.collective_compute(
                            "AllReduce",
                            ADD,
                            replica_groups=[[0, 1, 2, 3], [4, 5, 6, 7]],
                            ins=[kv_in_d.ap().opt()],
                            outs=[kv_out_d.ap().opt()],
                        )

                # kvblk: load reduced Grams, cast to fp16 block-diag
                kvblk = kvblk_pool.tile([128, NPAIR * 128], F16, tag="kvblk")
                if "C" in phases:
                    kvf = kvblk_pool.tile([128, NPAIR * 128], F32, tag="kvf")
                    nc.scalar.dma_start(out=kvf[:], in_=kv_out_d.ap())
                    nc.vector.memset(kvblk[:], 0.0)
                    # top-left diag blocks of each pair, then bottom-right
                    nc.vector.tensor_copy(
                        kvblk[0:64, :].rearrange("p (t f) -> p t f", t=NPAIR)[
                            :, :, 0:64
                        ],
                        kvf[0:64, :].rearrange("p (t f) -> p t f", t=NPAIR)[
                            :, :, 0:64
                        ],
                    )
                    nc.vector.tensor_copy(
                        kvblk[64:128, :].rearrange("p (t f) -> p t f", t=NPAIR)[
                            :, :, 64:128
                        ],
                        kvf[64:128, :].rearrange("p (t f) -> p t f", t=NPAIR)[
                            :, :, 64:128
                        ],
                    )

                # ==== Fused phase B+C: q proj/norm/rope + attn + out proj ===
                with ExitStack() as ctxB:
                  if "B" in phases and "C" in phases:
                    psB = ctxB.enter_context(
                        tc.tile_pool(name="psB", bufs=2, space="PSUM")
                    )
                    psN = ctxB.enter_context(
                        tc.tile_pool(name="psN", bufs=1, space="PSUM")
                    )
                    psAt = ctxB.enter_context(
                        tc.tile_pool(name="psAt", bufs=1, space="PSUM")
                    )
                    psO = ctxB.enter_context(
                        tc.tile_pool(name="psO", bufs=2, space="PSUM")
                    )
                    sbB = ctxB.enter_context(tc.tile_pool(name="sbB", bufs=3))
                    sbS = ctxB.enter_context(
                        tc.tile_pool(name="sbS", bufs=2 * NJ)
                    )
                    sbQ = ctxB.enter_context(tc.tile_pool(name="sbQ", bufs=2))
                    sbAt = ctxB.enter_context(
                        tc.tile_pool(name="sbAt", bufs=NJ + 2)
                    )

                    def _emit_attn_out(item):
                        ct_, qh_ = item
                        cs_ = slice(ct_ * 512, (ct_ + 1) * 512)
                        attn_sb = []
                        for hp in range(NPAIR):
                            a_ps = psAt.tile([128, 512], F32, tag="a_ps")
                            nc.tensor.matmul(
                                a_ps[:],
                                kvblk[:, hp * 128 : (hp + 1) * 128],
                                qh_[:, hp * 512 : (hp + 1) * 512],
                                start=True,
                                stop=True,
                            )
                            a_sb = sbAt.tile([128, 512], F16, tag="a_sb")
                            if hp % 2 == 0:
                                nc.scalar.activation(a_sb[:], a_ps[:], Copy)
                            else:
                                nc.vector.tensor_copy(a_sb[:], a_ps[:])
                            attn_sb.append(a_sb)

                        o_all = sbQ.tile([128, NJ * 512], F32, tag="o_all")
                        for et in range(NJ):
                            elo = et * 128
                            o_ps = psO.tile([128, 512], F32, tag="o_ps")
                            for jt in range(NJ):
                                nc.tensor.matmul(
                                    o_ps[:],
                                    wo_all[
                                        :, jt * DIM + elo : jt * DIM + elo + 128
                                    ],
                                    attn_sb[jt][:],
                                    start=(jt == 0),
                                    stop=(jt == NJ - 1),
                                )
                            nc.scalar.activation(
                                o_all[:, et * 512 : (et + 1) * 512], o_ps[:], Copy
                            )
                        nc.scalar.dma_start(
                            out=blkview(out_d, cs_),
                            in_=o_all[:].rearrange("p (t c) -> p t c", t=NJ),
                        )

                    at_pending = []
                    for ct in range(NQ_T):
                        cs = slice(ct * 512, (ct + 1) * 512)
                        norms_ps = psN.tile([16, 512], F32, tag="norms")
                        qh_all = sbQ.tile([128, NJ * 512], F16, tag="qhall")
                        q_sbs = []
                        # pass 1: projections + squares + norm accumulation
                        for jt in range(NJ):
                            jlo = jt * 128
                            q_ps = psB.tile([128, 512], F32, tag="q_ps")
                            for dc in range(ND):
                                nc.tensor.matmul(
                                    q_ps[:],
                                    wq_all[
                                        :, dc * DIM + jlo : dc * DIM + jlo + 128
                                    ],
                                    xsl(dc, cs),
                                    start=(dc == 0),
                                    stop=(dc == ND - 1),
                                )
                            q_sb = sbS.tile([128, 512], F16, tag="q_sb")
                            nc.scalar.activation(q_sb[:], q_ps[:], Copy)
                            sq = sbB.tile([128, 512], F16, tag="sqB")
                            nc.vector.tensor_mul(sq[:], q_sb[:], q_sb[:])
                            nc.tensor.matmul(
                                norms_ps[:],
                                ind16T_t[:, jt * 16 : (jt + 1) * 16],
                                sq[:],
                                start=(jt == 0),
                                stop=(jt == NJ - 1),
                            )
                            q_sbs.append(q_sb)

                        lnn = sbB.tile([16, 512], F32, tag="lnn")
                        nc.scalar.activation(lnn[:], norms_ps[:], Ln)
                        rs16 = sbB.tile([16, 512], F16, tag="rs16")
                        nc.scalar.activation(rs16[:], lnn[:], Exp, scale=-0.5)

                        # pass 2: rotation + rope + scale into qh_all
                        for jt in range(NJ):
                            q_sb = q_sbs[jt]
                            rot_ps = psB.tile([128, 512], F32, tag="rotrep")
                            nc.tensor.matmul(
                                rot_ps[:], P_t[:], q_sb[:], start=True, stop=True
                            )
                            rep_ps = psB.tile([128, 512], F32, tag="rotrep")
                            nc.tensor.matmul(
                                rep_ps[:],
                                ind16_t[:, jt * 128 : (jt + 1) * 128],
                                rs16[:],
                                start=True,
                                stop=True,
                            )
                            t1 = sbB.tile([128, 512], F16, tag="t1")
                            nc.vector.tensor_tensor(
                                t1[:], q_sb[:], cosF_t[:, cs], MUL
                            )
                            t2 = sbB.tile([128, 512], F16, tag="t2")
                            nc.vector.tensor_tensor(
                                t2[:], rot_ps[:], sinF_t[:, cs], MUL
                            )
                            s = sbB.tile([128, 512], F16, tag="sB")
                            nc.vector.tensor_tensor(s[:], t1[:], t2[:], ADD)
                            nc.vector.tensor_tensor(
                                qh_all[:, jt * 512 : (jt + 1) * 512],
                                s[:],
                                rep_ps[:],
                                MUL,
                            )

                        at_pending.append((ct, qh_all))
                        if len(at_pending) > 1:
                            _emit_attn_out(at_pending.pop(0))

                    while at_pending:
                        _emit_attn_out(at_pending.pop(0))

    nc.compile()
    return nc


_NC_CACHE = None


def _get_nc():
    global _NC_CACHE
    if _NC_CACHE is None:
        _NC_CACHE = build_nc()
    return _NC_CACHE


def make_in_maps(x, mask, Wq, Wk, Wv, Wo, norm_const):
    x = np.asarray(x, np.float32)
    mask = np.asarray(mask)
    Wq = np.asarray(Wq, np.float32)
    Wk = np.asarray(Wk, np.float32)
    Wv = np.asarray(Wv, np.float32)
    Wo = np.asarray(Wo, np.float32)
    norm_const = np.asarray(norm_const, np.float32).reshape(H)

    sig = 1.0 / (1.0 + np.exp(-norm_const.astype(np.float64)))
    svec = np.float64(C) ** (-sig)  # [H]
    s_cols = np.repeat(svec, HD)  # [DIM]

    f16 = np.float16
    WkT = np.ascontiguousarray(Wk.T).astype(f16)
    WvT = np.ascontiguousarray((Wv * s_cols[:, None].astype(np.float32)).T).astype(
        f16
    )
    WqT = np.ascontiguousarray(Wq.T).astype(f16)
    WoT = np.ascontiguousarray(Wo.T).astype(f16)

    inv_freq = 1.0 / (
        ROPE_THETA ** (np.arange(0, HD, 2, dtype=np.float64) / HD)
    )  # [32]
    freq_of_j = np.repeat(inv_freq, 2)  # [64] interleaved

    ind16T = np.zeros((DIM, 16), f16)
    for jt in range(NJ):
        for kk in range(128):
            ind16T[jt * 128 + kk, 2 * jt + (kk >= 64)] = 1.0

    ind16 = np.zeros((16, DIM), f16)
    for jt in range(NJ):
        for m in range(128):
            ind16[2 * jt + (m >= 64), jt * 128 + m] = 1.0

    Pmat = np.zeros((128, 128), f16)
    for i in range(64):
        Pmat[2 * i + 1, 2 * i] = -1.0  # out[2i] = -q[2i+1]
        Pmat[2 * i, 2 * i + 1] = 1.0  # out[2i+1] = q[2i]

    in_maps = []
    for core in range(N_CORES):
        b = core // (N_CORES // B)
        cc = core % (N_CORES // B)
        c0 = cc * R
        pos = (c0 + np.arange(R)).astype(np.float64)

        xTc = np.ascontiguousarray(x[b, c0 : c0 + R, :].T).astype(f16)

        angC = pos[:, None] * freq_of_j[None, :]  # [R, 64]
        cosCc = np.cos(angC).astype(f16)
        sinCc = np.sin(angC).astype(np.float32)
        # sign fold for the swap formulation: even j -> -sin, odd j -> +sin
        sinCc[:, 0::2] *= -1.0
        sinCc = sinCc.astype(f16)

        angF = freq_of_j[:, None] * pos[None, :]  # [64, R]
        angF2 = np.concatenate([angF, angF], axis=0)  # [128, R]
        cosFc = np.cos(angF2).astype(f16)
        sinFc = np.sin(angF2).astype(f16)

        mrow = mask[b, c0 : c0 + R].astype(np.float32)  # [R]
        maskCc = np.ascontiguousarray(mrow.reshape(NC_T, 128).T)  # [128, NC_T]

        in_maps.append(
            {
                "xT": xTc,
                "WkT": WkT,
                "WvT": WvT,
                "WqT": WqT,
                "WoT": WoT,
                "cosC": cosCc,
                "sinC": sinCc,
                "cosF": cosFc,
                "sinF": sinFc,
                "maskC": maskCc,
                "ind16T": ind16T,
                "ind16": ind16,
                "Pmat": Pmat,
            }
        )
    return in_maps


def assemble_output(results, mask):
    out = np.empty((B, C, DIM), np.float32)
    for core in range(N_CORES):
        b = core // (N_CORES // B)
        cc = core % (N_CORES // B)
        c0 = cc * R
        out[b, c0 : c0 + R, :] = results[core]["out"].T
    # q-side mask: masked rows produce zero output
    out *= np.asarray(mask)[:, :, None].astype(np.float32)
    return out


def kernel(x, mask, Wq, Wk, Wv, Wo, norm_const):
    nc = _get_nc()
    in_maps = make_in_maps(x, mask, Wq, Wk, Wv, Wo, norm_const)
    res = run_bass_kernel_spmd(nc, in_maps, list(range(N_CORES)))
    return assemble_output(res.results, mask)



# revision 15
# speedup vs baseline: 3.2177x; 3.2177x over previous
"""Trainium2 Bass kernel for nn_Attention_43413529428606 (linear attention
with l2-normed q/k, interleaved RoPE, mask, per-head power scaling).

v2: mask compaction.  Only the ~50% unmasked rows are processed: the host
gathers rows with mask==1 per batch, splits them over the 4 cores of that
batch (cores 0-3 batch 0, 4-7 batch 1; capacity 1024 rows/core), and
scatters the output back (masked rows are zero by construction).  Each core
projects k/v for its rows, applies l2norm+RoPE, accumulates the per-head
transposed Gram state kvT = sum_c v ⊗ khat, AllReduces it in fp16 (256 KB)
within its batch group, folds Wo into the state (G = kvT^T @ WoT per head
pair), then computes q/norm/rope and the fused attn+out projection
out = qhat @ G for its rows.  Activation-engine usage is restricted to
{Copy, Rsqrt} (one table set), squares run on DVE, reductions and some
PSUM evictions on Pool, so every engine stays off the critical path of the
Tensor engine.

Self-contained: hardcodes all shapes; no sibling imports.
"""

import sys

for _p in ("/opt/trn_rl_repo",):
    if _p not in sys.path:
        sys.path.append(_p)

from contextlib import ExitStack

import numpy as np

import concourse.bass as bass
import concourse.bacc as bacc
import concourse.tile as tile
from concourse import mybir
from concourse.bass_utils import run_bass_kernel_spmd

F32 = mybir.dt.float32
F16 = mybir.dt.float16

DIM = 1024
H = 16
HD = 64
B = 2
C = 8192
ROPE_THETA = 10000.0

N_CORES = 8
R = 1024  # compacted rows per core (capacity; ~n_unmasked/4)
NC_T = R // 128  # 8 c-tiles of 128 (phase A)
NQ_T = R // 512  # 2 c-supertiles of 512 (phase B)
ND = DIM // 128  # 8 d-chunks
NJ = DIM // 128  # 8 j-tiles
NPAIR = H // 2  # 8 head pairs

Copy = mybir.ActivationFunctionType.Copy
Sqrt = mybir.ActivationFunctionType.Sqrt
MUL = mybir.AluOpType.mult
ADD = mybir.AluOpType.add


def build_nc(sim_mode=False, phases="ABC", reps=1, coll=True):
    nc = bacc.Bacc(
        "TRN2",
        target_bir_lowering=False,
        debug=False,
        num_devices=1 if sim_mode else N_CORES,
    )

    # ---- DRAM parameters (per-core shapes, fp16 data path) ----
    xT = nc.dram_tensor("xT", [DIM, R], F16, kind="ExternalInput").ap()
    WkT = nc.dram_tensor("WkT", [DIM, DIM], F16, kind="ExternalInput").ap()
    WvT = nc.dram_tensor("WvT", [DIM, DIM], F16, kind="ExternalInput").ap()
    WqT = nc.dram_tensor("WqT", [DIM, DIM], F16, kind="ExternalInput").ap()
    WoT = nc.dram_tensor("WoT", [DIM, DIM], F16, kind="ExternalInput").ap()
    cosC = nc.dram_tensor("cosC", [R, HD], F16, kind="ExternalInput").ap()
    sinC = nc.dram_tensor("sinC", [R, HD], F16, kind="ExternalInput").ap()
    cosF = nc.dram_tensor("cosF", [128, R], F16, kind="ExternalInput").ap()
    sinF = nc.dram_tensor("sinF", [128, R], F16, kind="ExternalInput").ap()
    maskC = nc.dram_tensor("maskC", [128, NC_T], F32, kind="ExternalInput").ap()
    ind16T = nc.dram_tensor("ind16T", [DIM, 16], F16, kind="ExternalInput").ap()
    ind16 = nc.dram_tensor("ind16", [16, DIM], F16, kind="ExternalInput").ap()
    Pmat = nc.dram_tensor("Pmat", [128, 128], F16, kind="ExternalInput").ap()

    kv_in_d = nc.dram_tensor("kv_in_d", [128, NPAIR * 128], F16)
    kv_out_d = nc.dram_tensor("kv_out_d", [128, NPAIR * 128], F16)

    out_d = nc.dram_tensor("out", [DIM, R], F16, kind="ExternalOutput").ap()

    def blkview(dram_ap, csl):
        return dram_ap.rearrange("(t p) c -> p t c", p=128)[:, :, csl]

    with tile.TileContext(nc) as tc:
        with ExitStack() as ctx:
            consts = ctx.enter_context(tc.tile_pool(name="consts", bufs=1))

            cosC_t = consts.tile([128, NC_T * HD], F16, tag="cosC")
            sinC_t = consts.tile([128, NC_T * HD], F16, tag="sinC")
            nc.sync.dma_start(
                out=cosC_t[:].rearrange("p (t f) -> p t f", t=NC_T),
                in_=cosC[:].rearrange("(t p) f -> p t f", p=128),
            )
            nc.sync.dma_start(
                out=sinC_t[:].rearrange("p (t f) -> p t f", t=NC_T),
                in_=sinC[:].rearrange("(t p) f -> p t f", p=128),
            )
            maskC_t = consts.tile([128, NC_T], F32, tag="maskC")
            ind16T_t = consts.tile([128, NJ * 16], F16, tag="ind16T")
            ind16_t = consts.tile([16, DIM], F16, tag="ind16")
            P_t = consts.tile([128, 128], F16, tag="Pmat")
            nc.sync.dma_start(out=maskC_t[:], in_=maskC[:])
            nc.sync.dma_start(
                out=ind16T_t[:].rearrange("p (t f) -> p t f", t=NJ),
                in_=ind16T[:].rearrange("(t p) f -> p t f", p=128),
            )
            nc.sync.dma_start(out=ind16_t[:], in_=ind16[:])
            nc.sync.dma_start(out=P_t[:], in_=Pmat[:])

            for _rep in range(reps):
              with ExitStack() as ctxX:
                xpool = ctxX.enter_context(tc.tile_pool(name="xpool", bufs=1))
                xT_all = xpool.tile([128, ND * R], F16, tag="xT")
                # column-chunked x load: 2 c-tiles per DMA so phase A can
                # start before the whole x lands
                for xc in range(4):
                    csl = slice(xc * 256, (xc + 1) * 256)
                    nc.sync.dma_start(
                        out=xT_all[:]
                        .rearrange("p (t c) -> p t c", t=ND)[:, :, csl],
                        in_=xT[:, csl].rearrange("(t p) c -> p t c", p=128),
                    )

                def xsl(dc, csl):
                    lo = dc * R
                    return xT_all[:, lo + csl.start : lo + csl.stop]

                if "B" in phases and "C" in phases:
                    wBC = ctxX.enter_context(tc.tile_pool(name="wBC", bufs=1))
                    wq_all = wBC.tile([128, ND * DIM], F16, tag="wq")
                    nc.sync.dma_start(
                        out=wq_all[:].rearrange("p (t f) -> p t f", t=ND),
                        in_=WqT[:].rearrange("(t p) f -> p t f", p=128),
                    )
                    wo_all = wBC.tile([128, ND * DIM], F16, tag="wo")
                    nc.scalar.dma_start(
                        out=wo_all[:].rearrange("p (t f) -> p t f", t=ND),
                        in_=WoT[:].rearrange("(t p) f -> p t f", p=128),
                    )
                    cosF_t = wBC.tile([128, R], F16, tag="cosF")
                    sinF_t = wBC.tile([128, R], F16, tag="sinF")
                    nc.sync.dma_start(out=cosF_t[:], in_=cosF[:])
                    nc.scalar.dma_start(out=sinF_t[:], in_=sinF[:])
                    G_pool = ctxX.enter_context(
                        tc.tile_pool(name="G_pool", bufs=1)
                    )
                    G_all = G_pool.tile([128, NJ * DIM], F16, tag="G")

                # ========= Phase A: k/v proj + process + kvT Grams ==========
                with ExitStack() as ctxA:
                  if "A" in phases:
                    wA = ctxA.enter_context(tc.tile_pool(name="wA", bufs=1))
                    psA = ctxA.enter_context(
                        tc.tile_pool(name="psA", bufs=3, space="PSUM")
                    )
                    pskv = ctxA.enter_context(
                        tc.tile_pool(name="pskv", bufs=1, space="PSUM")
                    )
                    sbA = ctxA.enter_context(tc.tile_pool(name="sbA", bufs=2))
                    sb1 = ctxA.enter_context(tc.tile_pool(name="sb1", bufs=2))
                    smA = ctxA.enter_context(tc.tile_pool(name="smA", bufs=2))

                    wk_all = wA.tile([128, ND * DIM], F16, tag="wk")
                    wv_all = wA.tile([128, ND * DIM], F16, tag="wv")
                    for wt, wsrc in ((wk_all, WkT), (wv_all, WvT)):
                        for xc in range(2):
                            nc.scalar.dma_start(
                                out=wt[
                                    :, xc * 4 * DIM : (xc + 1) * 4 * DIM
                                ].rearrange("p (t f) -> p t f", t=4),
                                in_=wsrc[xc * 512 : (xc + 1) * 512, :].rearrange(
                                    "(t p) f -> p t f", p=128
                                ),
                            )

                    kv_ps = pskv.tile([128, NPAIR * 128], F32, tag="kvps")
                    kv_pending = []

                    # kvT Gram: out[e,d] = sum_c v[c,e] khat[c,d]
                    # On HW start=True zeroes the whole PSUM bank, so only
                    # the first pair written to each bank may carry it.
                    def _emit_kv(item):
                        ct_, khat_, v_ = item
                        for p in range(NPAIR):
                            ps_ = slice(p * 128, (p + 1) * 128)
                            nc.tensor.matmul(
                                kv_ps[:, ps_],
                                v_[:, ps_],
                                khat_[:, ps_],
                                start=(
                                    True
                                    if sim_mode
                                    else (ct_ == 0 and p % 4 == 0)
                                ),
                                stop=(
                                    True if sim_mode else (ct_ == NC_T - 1)
                                ),
                            )

                    for ct in range(NC_T):
                        cs = slice(ct * 128, (ct + 1) * 128)
                        k_ps = psA.tile([128, DIM], F32, tag="proj_ps")
                        v_ps = psA.tile([128, DIM], F32, tag="proj_ps")
                        for half in range(2):
                            js = slice(half * 512, (half + 1) * 512)
                            for dc in range(ND):
                                nc.tensor.matmul(
                                    k_ps[:, js],
                                    xsl(dc, cs),
                                    wk_all[
                                        :, dc * DIM + js.start : dc * DIM + js.stop
                                    ],
                                    start=(dc == 0),
                                    stop=(dc == ND - 1),
                                )
                            for dc in range(ND):
                                nc.tensor.matmul(
                                    v_ps[:, js],
                                    xsl(dc, cs),
                                    wv_all[
                                        :, dc * DIM + js.start : dc * DIM + js.stop
                                    ],
                                    start=(dc == 0),
                                    stop=(dc == ND - 1),
                                )

                        # evictions: v and k on Act (cast fp16)
                        v_sb = sbA.tile([128, DIM], F16, tag="v_sb")
                        nc.scalar.activation(v_sb[:], v_ps[:], Copy)
                        k_sb = sbA.tile([128, DIM], F16, tag="k_sb")
                        nc.scalar.activation(k_sb[:], k_ps[:], Copy)
                        # squares for the l2 norm (Pool, SBUF-only)
                        sq = sbA.tile([128, DIM], F16, tag="sq")
                        nc.gpsimd.tensor_mul(sq[:], k_sb[:], k_sb[:])

                        cosb = (
                            cosC_t[:, ct * HD : (ct + 1) * HD]
                            .unsqueeze(1)
                            .broadcast_to([128, H, HD])
                        )
                        sinb4 = (
                            sinC_t[:, ct * HD : (ct + 1) * HD]
                            .rearrange("p (g two) -> p g two", two=2)
                            .unsqueeze(1)
                            .broadcast_to([128, H, HD // 2, 2])
                        )
                        k3 = k_sb[:].rearrange("p (h f) -> p h f", h=H)
                        k_sw = k_sb[:].rearrange(
                            "p (h g two) -> p h g two", h=H, two=2
                        )[:, :, :, ::-1]

                        m1 = sb1.tile([128, DIM], F16, tag="m1")
                        nc.vector.tensor_tensor(
                            m1[:].rearrange("p (h f) -> p h f", h=H), k3, cosb, MUL
                        )
                        red = smA.tile([128, H], F32, tag="red")
                        nc.vector.tensor_reduce(
                            red[:],
                            sq[:].rearrange("p (h f) -> p h f", h=H),
                            mybir.AxisListType.X,
                            ADD,
                        )
                        srt = smA.tile([128, H], F32, tag="srt")
                        nc.scalar.activation(srt[:], red[:], Sqrt)
                        rs = smA.tile([128, H], F32, tag="rs")
                        nc.vector.reciprocal(rs[:], srt[:])
                        rsm = smA.tile([128, H], F16, tag="rsm")
                        nc.vector.tensor_scalar_mul(
                            rsm[:], rs[:], maskC_t[:, ct : ct + 1]
                        )
                        m2 = sb1.tile([128, DIM], F16, tag="m2")
                        nc.vector.tensor_tensor(
                            m2[:].rearrange("p (h g two) -> p h g two", h=H, two=2),
                            k_sw,
                            sinb4,
                            MUL,
                        )
                        s = sb1.tile([128, DIM], F16, tag="s")
                        nc.gpsimd.tensor_tensor(s[:], m1[:], m2[:], ADD)
                        khat = sbA.tile([128, DIM], F16, tag="khat")
                        rsb = rsm[:].unsqueeze(2).broadcast_to([128, H, HD])
                        nc.vector.tensor_tensor(
                            khat[:].rearrange("p (h f) -> p h f", h=H),
                            s[:].rearrange("p (h f) -> p h f", h=H),
                            rsb,
                            MUL,
                        )

                        # kvT Grams are issued one iteration late (software
                        # pipelining) so PE never waits on the khat chain
                        kv_pending.append((ct, khat, v_sb))
                        if len(kv_pending) > 1:
                            _emit_kv(kv_pending.pop(0))

                    while kv_pending:
                        _emit_kv(kv_pending.pop(0))

                    # evict kvT partials (fp16) and run the collective
                    kv_sb = sbA.tile([128, NPAIR * 128], F16, tag="kv_sb")
                    nc.vector.tensor_copy(kv_sb[:], kv_ps[:])
                    nc.sync.dma_start(out=kv_in_d.ap(), in_=kv_sb[:])
                    if sim_mode or not coll:
                        nc.sync.dma_start(out=kv_out_d.ap(), in_=kv_in_d.ap())
                    else:
                        nc.gpsimd.collective_compute(
                            "AllReduce",
                            ADD,
                            replica_groups=[[0, 1, 2, 3], [4, 5, 6, 7]],
                            ins=[kv_in_d.ap().opt()],
                            outs=[kv_out_d.ap().opt()],
                        )

                # ==== Fused phase B+C: q proj/norm/rope + (attn·Wo) ===
                with ExitStack() as ctxB:
                  if "B" in phases and "C" in phases:
                    psB = ctxB.enter_context(
                        tc.tile_pool(name="psB", bufs=2, space="PSUM")
                    )
                    psN = ctxB.enter_context(
                        tc.tile_pool(name="psN", bufs=1, space="PSUM")
                    )
                    psR = ctxB.enter_context(
                        tc.tile_pool(name="psR", bufs=2, space="PSUM")
                    )
                    psO = ctxB.enter_context(
                        tc.tile_pool(name="psO", bufs=2, space="PSUM")
                    )
                    sbB = ctxB.enter_context(tc.tile_pool(name="sbB", bufs=3))
                    sbS = ctxB.enter_context(
                        tc.tile_pool(name="sbS", bufs=2 * NJ)
                    )
                    sbQ = ctxB.enter_context(
                        tc.tile_pool(name="sbQ", bufs=2 * NQ_T)
                    )

                    def _emit_pass1(ct_):
                        """q proj + squares + norm accumulation; returns
                        (q_sbs, norms_ps)."""
                        cs = slice(ct_ * 512, (ct_ + 1) * 512)
                        norms_ps = psN.tile([16, 512], F32, tag="norms")
                        q_sbs = []
                        for jt in range(NJ):
                            jlo = jt * 128
                            q_ps = psB.tile([128, 512], F32, tag="q_ps")
                            for dc in range(ND):
                                nc.tensor.matmul(
                                    q_ps[:],
                                    wq_all[
                                        :, dc * DIM + jlo : dc * DIM + jlo + 128
                                    ],
                                    xsl(dc, cs),
                                    start=(dc == 0),
                                    stop=(dc == ND - 1),
                                )
                            q_sb = sbS.tile([128, 512], F16, tag="q_sb")
                            nc.scalar.activation(q_sb[:], q_ps[:], Copy)
                            sq = sbB.tile([128, 512], F16, tag="sqB")
                            nc.vector.tensor_mul(sq[:], q_sb[:], q_sb[:])
                            nc.tensor.matmul(
                                norms_ps[:],
                                ind16T_t[:, jt * 16 : (jt + 1) * 16],
                                sq[:],
                                start=(jt == 0),
                                stop=(jt == NJ - 1),
                            )
                            q_sbs.append(q_sb)
                        return q_sbs, norms_ps

                    def _emit_pass2(ct_, q_sbs, norms_ps):
                        """rsqrt + rope + scale; returns qh_all."""
                        cs = slice(ct_ * 512, (ct_ + 1) * 512)
                        nsrt = sbB.tile([16, 512], F32, tag="nsrt")
                        nc.scalar.activation(nsrt[:], norms_ps[:], Sqrt)
                        nrcp = sbB.tile([16, 512], F32, tag="nrcp")
                        nc.vector.reciprocal(nrcp[:], nsrt[:])
                        rs16 = sbB.tile([16, 512], F16, tag="rs16")
                        nc.vector.tensor_copy(rs16[:], nrcp[:])
                        qh_all = sbQ.tile([128, NJ * 512], F16, tag="qhall")
                        for jt in range(NJ):
                            q_sb = q_sbs[jt]
                            rot_ps = psR.tile([128, 512], F32, tag="rotrep")
                            nc.tensor.matmul(
                                rot_ps[:], P_t[:], q_sb[:], start=True, stop=True
                            )
                            rep_ps = psR.tile([128, 512], F32, tag="rotrep")
                            nc.tensor.matmul(
                                rep_ps[:],
                                ind16_t[:, jt * 128 : (jt + 1) * 128],
                                rs16[:],
                                start=True,
                                stop=True,
                            )
                            rot_sb = sbB.tile([128, 512], F16, tag="rot_sb")
                            nc.scalar.activation(rot_sb[:], rot_ps[:], Copy)
                            rep_sb = sbB.tile([128, 512], F16, tag="rep_sb")
                            nc.vector.tensor_copy(rep_sb[:], rep_ps[:])
                            t1 = sbB.tile([128, 512], F16, tag="t1")
                            nc.vector.tensor_tensor(
                                t1[:], q_sb[:], cosF_t[:, cs], MUL
                            )
                            t2 = sbB.tile([128, 512], F16, tag="t2")
                            nc.vector.tensor_tensor(
                                t2[:], rot_sb[:], sinF_t[:, cs], MUL
                            )
                            s = sbB.tile([128, 512], F16, tag="sB")
                            nc.vector.tensor_tensor(s[:], t1[:], t2[:], ADD)
                            nc.vector.tensor_tensor(
                                qh_all[:, jt * 512 : (jt + 1) * 512],
                                s[:],
                                rep_sb[:],
                                MUL,
                            )
                        return qh_all

                    def _emit_G():
                        """kvT load (blocks on collective) -> G = kvT^T@WoT."""
                        kvpool = ctxB.enter_context(
                            tc.tile_pool(name="kvpool", bufs=1)
                        )
                        kvf = kvpool.tile([128, NPAIR * 128], F16, tag="kvf")
                        nc.scalar.dma_start(out=kvf[:], in_=kv_out_d.ap())
                        kvblk = kvpool.tile([128, NPAIR * 128], F16, tag="kvblk")
                        nc.vector.memset(kvblk[:], 0.0)
                        nc.vector.tensor_copy(
                            kvblk[0:64, :].rearrange(
                                "p (t f) -> p t f", t=NPAIR
                            )[:, :, 0:64],
                            kvf[0:64, :].rearrange("p (t f) -> p t f", t=NPAIR)[
                                :, :, 0:64
                            ],
                        )
                        nc.vector.tensor_copy(
                            kvblk[64:128, :].rearrange(
                                "p (t f) -> p t f", t=NPAIR
                            )[:, :, 64:128],
                            kvf[64:128, :].rearrange(
                                "p (t f) -> p t f", t=NPAIR
                            )[:, :, 64:128],
                        )
                        for hp in range(NPAIR):
                            for half in range(2):
                                fs = slice(hp * DIM + half * 512,
                                           hp * DIM + (half + 1) * 512)
                                g_ps = psO.tile([128, 512], F32, tag="o_ps")
                                nc.tensor.matmul(
                                    g_ps[:],
                                    kvblk[:, hp * 128 : (hp + 1) * 128],
                                    wo_all[:, fs],
                                    start=True,
                                    stop=True,
                                )
                                nc.scalar.activation(
                                    G_all[:, fs], g_ps[:], Copy
                                )

                    def _emit_out(ct_, qh_):
                        cs_ = slice(ct_ * 512, (ct_ + 1) * 512)
                        o_all = sbQ.tile([128, NJ * 512], F16, tag="o_all")
                        for et in range(NJ):
                            elo = et * 128
                            o_ps = psO.tile([128, 512], F32, tag="o_ps")
                            for jt in range(NJ):
                                nc.tensor.matmul(
                                    o_ps[:],
                                    G_all[
                                        :, jt * DIM + elo : jt * DIM + elo + 128
                                    ],
                                    qh_[:, jt * 512 : (jt + 1) * 512],
                                    start=(jt == 0),
                                    stop=(jt == NJ - 1),
                                )
                            nc.scalar.activation(
                                o_all[:, et * 512 : (et + 1) * 512], o_ps[:], Copy
                            )
                        nc.scalar.dma_start(
                            out=blkview(out_d, cs_),
                            in_=o_all[:].rearrange("p (t c) -> p t c", t=NJ),
                        )

                    # emission order: pass1(0), pass2(0), pass1(1), G,
                    # pass2(1), out(0), out(1) — G sits behind ~45us of
                    # independent PE work to hide the AllReduce latency.
                    q0, n0 = _emit_pass1(0)
                    qh0 = _emit_pass2(0, q0, n0)
                    q1, n1 = _emit_pass1(1)
                    _emit_G()
                    qh1 = _emit_pass2(1, q1, n1)
                    _emit_out(0, qh0)
                    _emit_out(1, qh1)

    nc.compile()
    return nc


_NC_CACHE = None


def _get_nc():
    global _NC_CACHE
    if _NC_CACHE is None:
        _NC_CACHE = build_nc()
    return _NC_CACHE


def _plan_rows(mask):
    """Split each batch's unmasked row indices over its 4 cores.
    Returns rows[core] = np.ndarray of original row ids (len <= R)."""
    mask = np.asarray(mask)
    rows = []
    for b in range(B):
        idx = np.flatnonzero(mask[b] != 0)
        n = len(idx)
        assert n <= 4 * R, f"unmasked rows {n} exceed capacity {4 * R}"
        szs = [n // 4 + (1 if i < n % 4 else 0) for i in range(4)]
        off = 0
        for cc in range(4):
            rows.append(idx[off : off + szs[cc]])
            off += szs[cc]
    return rows


def make_in_maps(x, mask, Wq, Wk, Wv, Wo, norm_const):
    x = np.asarray(x, np.float32)
    mask = np.asarray(mask)
    Wq = np.asarray(Wq, np.float32)
    Wk = np.asarray(Wk, np.float32)
    Wv = np.asarray(Wv, np.float32)
    Wo = np.asarray(Wo, np.float32)
    norm_const = np.asarray(norm_const, np.float32).reshape(H)

    sig = 1.0 / (1.0 + np.exp(-norm_const.astype(np.float64)))
    svec = np.float64(C) ** (-sig)  # [H]
    s_cols = np.repeat(svec, HD)  # [DIM]

    f16 = np.float16
    WkT = np.ascontiguousarray(Wk.T).astype(f16)
    WvT = np.ascontiguousarray((Wv * s_cols[:, None].astype(np.float32)).T).astype(
        f16
    )
    WqT = np.ascontiguousarray(Wq.T).astype(f16)
    WoT = np.ascontiguousarray(Wo.T).astype(f16)

    inv_freq = 1.0 / (
        ROPE_THETA ** (np.arange(0, HD, 2, dtype=np.float64) / HD)
    )  # [32]
    freq_of_j = np.repeat(inv_freq, 2)  # [64] interleaved

    ind16T = np.zeros((DIM, 16), f16)
    for jt in range(NJ):
        for kk in range(128):
            ind16T[jt * 128 + kk, 2 * jt + (kk >= 64)] = 1.0

    ind16 = np.zeros((16, DIM), f16)
    for jt in range(NJ):
        for m in range(128):
            ind16[2 * jt + (m >= 64), jt * 128 + m] = 1.0

    Pmat = np.zeros((128, 128), f16)
    for i in range(64):
        Pmat[2 * i + 1, 2 * i] = -1.0  # out[2i] = -q[2i+1]
        Pmat[2 * i, 2 * i + 1] = 1.0  # out[2i+1] = q[2i]

    rows_per_core = _plan_rows(mask)

    in_maps = []
    for core in range(N_CORES):
        b = core // (N_CORES // B)
        rows = rows_per_core[core]
        sz = len(rows)
        pos = np.zeros(R, np.float64)
        pos[:sz] = rows.astype(np.float64)

        # gathered x columns; pads filled with 1.0 (khat row is zeroed by
        # the mask so pads contribute nothing)
        xTc = np.ones((DIM, R), f16)
        xTc[:, :sz] = x[b, rows, :].T.astype(f16)

        angC = pos[:, None] * freq_of_j[None, :]  # [R, 64]
        cosCc = np.cos(angC).astype(f16)
        sinCc = np.sin(angC).astype(np.float32)
        # sign fold for the swap formulation: even j -> -sin, odd j -> +sin
        sinCc[:, 0::2] *= -1.0
        sinCc = sinCc.astype(f16)

        angF = freq_of_j[:, None] * pos[None, :]  # [64, R]
        angF2 = np.concatenate([angF, angF], axis=0)  # [128, R]
        cosFc = np.cos(angF2).astype(f16)
        sinFc = np.sin(angF2).astype(f16)

        mrow = np.zeros(R, np.float32)
        mrow[:sz] = 1.0
        maskCc = np.ascontiguousarray(mrow.reshape(NC_T, 128).T)  # [128, NC_T]

        in_maps.append(
            {
                "xT": xTc,
                "WkT": WkT,
                "WvT": WvT,
                "WqT": WqT,
                "WoT": WoT,
                "cosC": cosCc,
                "sinC": sinCc,
                "cosF": cosFc,
                "sinF": sinFc,
                "maskC": maskCc,
                "ind16T": ind16T,
                "ind16": ind16,
                "Pmat": Pmat,
            }
        )
    return in_maps


def assemble_output(results, mask):
    rows_per_core = _plan_rows(mask)
    out = np.zeros((B, C, DIM), np.float32)
    for core in range(N_CORES):
        b = core // (N_CORES // B)
        rows = rows_per_core[core]
        sz = len(rows)
        out[b, rows, :] = results[core]["out"].T[:sz].astype(np.float32)
    return out


def kernel(x, mask, Wq, Wk, Wv, Wo, norm_const):
    nc = _get_nc()
    in_maps = make_in_maps(x, mask, Wq, Wk, Wv, Wo, norm_const)
    res = run_bass_kernel_spmd(nc, in_maps, list(range(N_CORES)))
    return assemble_output(res.results, mask)


# revision 33
# speedup vs baseline: 3.4019x; 1.0573x over previous
"""Trainium2 Bass kernel for nn_Attention_43413529428606 (linear attention
with l2-normed q/k, interleaved RoPE, mask, per-head power scaling).

v2: mask compaction.  Only the ~50% unmasked rows are processed: the host
gathers rows with mask==1 per batch, splits them over the 4 cores of that
batch (cores 0-3 batch 0, 4-7 batch 1; capacity 1024 rows/core), and
scatters the output back (masked rows are zero by construction).  Each core
projects k/v for its rows, applies l2norm+RoPE, accumulates the per-head
transposed Gram state kvT = sum_c v ⊗ khat, AllReduces it in fp16 (256 KB)
within its batch group, folds Wo into the state (G = kvT^T @ WoT per head
pair), then computes q/norm/rope and the fused attn+out projection
out = qhat @ G for its rows.  Activation-engine usage is restricted to
{Copy, Rsqrt} (one table set), squares run on DVE, reductions and some
PSUM evictions on Pool, so every engine stays off the critical path of the
Tensor engine.

Self-contained: hardcodes all shapes; no sibling imports.
"""

import sys

for _p in ("/opt/trn_rl_repo",):
    if _p not in sys.path:
        sys.path.append(_p)

from contextlib import ExitStack

import numpy as np

import concourse.bass as bass
import concourse.bacc as bacc
import concourse.tile as tile
from concourse import mybir
from concourse.bass_utils import run_bass_kernel_spmd

F32 = mybir.dt.float32
F16 = mybir.dt.float16

DIM = 1024
H = 16
HD = 64
B = 2
C = 8192
ROPE_THETA = 10000.0

N_CORES = 8
R = 1024  # compacted rows per core (capacity; ~n_unmasked/4)
NC_T = R // 128  # 8 c-tiles of 128 (phase A)
NQ_T = R // 512  # 2 c-supertiles of 512 (phase B)
ND = DIM // 128  # 8 d-chunks
NJ = DIM // 128  # 8 j-tiles
NPAIR = H // 2  # 8 head pairs

Copy = mybir.ActivationFunctionType.Copy
Sqrt = mybir.ActivationFunctionType.Sqrt
MUL = mybir.AluOpType.mult
ADD = mybir.AluOpType.add


def build_nc(sim_mode=False, phases="ABC", reps=1, coll=True):
    nc = bacc.Bacc(
        "TRN2",
        target_bir_lowering=False,
        debug=False,
        num_devices=1 if sim_mode else N_CORES,
    )

    # ---- DRAM parameters (per-core shapes, fp16 data path) ----
    xT = nc.dram_tensor("xT", [DIM, R], F16, kind="ExternalInput").ap()
    WkT = nc.dram_tensor("WkT", [DIM, DIM], F16, kind="ExternalInput").ap()
    WvT = nc.dram_tensor("WvT", [DIM, DIM], F16, kind="ExternalInput").ap()
    WqT = nc.dram_tensor("WqT", [DIM, DIM], F16, kind="ExternalInput").ap()
    WoT = nc.dram_tensor("WoT", [DIM, DIM], F16, kind="ExternalInput").ap()
    cosC = nc.dram_tensor("cosC", [R, HD], F16, kind="ExternalInput").ap()
    sinC = nc.dram_tensor("sinC", [R, HD], F16, kind="ExternalInput").ap()
    cosF = nc.dram_tensor("cosF", [128, R], F16, kind="ExternalInput").ap()
    sinF = nc.dram_tensor("sinF", [128, R], F16, kind="ExternalInput").ap()
    maskC = nc.dram_tensor("maskC", [128, NC_T], F32, kind="ExternalInput").ap()
    ind16T = nc.dram_tensor("ind16T", [DIM, 16], F16, kind="ExternalInput").ap()
    ind16 = nc.dram_tensor("ind16", [16, DIM], F16, kind="ExternalInput").ap()
    Pmat = nc.dram_tensor("Pmat", [128, 128], F16, kind="ExternalInput").ap()

    kv_in_d = nc.dram_tensor("kv_in_d", [128, NPAIR * 128], F16)
    kv_out_d = nc.dram_tensor("kv_out_d", [128, NPAIR * 128], F16)

    out_d = nc.dram_tensor("out", [DIM, R], F16, kind="ExternalOutput").ap()

    def blkview(dram_ap, csl):
        return dram_ap.rearrange("(t p) c -> p t c", p=128)[:, :, csl]

    with tile.TileContext(nc) as tc:
        with ExitStack() as ctx:
            consts = ctx.enter_context(tc.tile_pool(name="consts", bufs=1))

            cosC_t = consts.tile([128, NC_T * HD], F16, tag="cosC")
            sinC_t = consts.tile([128, NC_T * HD], F16, tag="sinC")
            maskC_t = consts.tile([128, NC_T], F32, tag="maskC")
            ind16T_t = consts.tile([128, NJ * 16], F16, tag="ind16T")
            ind16_t = consts.tile([16, DIM], F16, tag="ind16")
            P_t = consts.tile([128, 128], F16, tag="Pmat")
            consts_emitted = [False]

            def _emit_consts():
                if consts_emitted[0]:
                    return
                consts_emitted[0] = True
                nc.sync.dma_start(
                    out=cosC_t[:].rearrange("p (t f) -> p t f", t=NC_T),
                    in_=cosC[:].rearrange("(t p) f -> p t f", p=128),
                )
                nc.sync.dma_start(
                    out=sinC_t[:].rearrange("p (t f) -> p t f", t=NC_T),
                    in_=sinC[:].rearrange("(t p) f -> p t f", p=128),
                )
                nc.sync.dma_start(out=maskC_t[:], in_=maskC[:])
                nc.sync.dma_start(
                    out=ind16T_t[:].rearrange("p (t f) -> p t f", t=NJ),
                    in_=ind16T[:].rearrange("(t p) f -> p t f", p=128),
                )
                nc.sync.dma_start(out=ind16_t[:], in_=ind16[:])
                nc.sync.dma_start(out=P_t[:], in_=Pmat[:])

            for _rep in range(reps):
              with ExitStack() as ctxX:
                xpool = ctxX.enter_context(tc.tile_pool(name="xpool", bufs=1))
                xT_all = xpool.tile([128, ND * R], F16, tag="xT")

                def _xdma(xc):
                    csl = slice(xc * 256, (xc + 1) * 256)
                    nc.sync.dma_start(
                        out=xT_all[:]
                        .rearrange("p (t c) -> p t c", t=ND)[:, :, csl],
                        in_=xT[:, csl].rearrange("(t p) c -> p t c", p=128),
                    )

                def _wdma(wt, wsrc, jc):
                    jsl = slice(jc * 512, (jc + 1) * 512)
                    nc.sync.dma_start(
                        out=wt[:].rearrange("p (t f) -> p t f", t=ND)[:, :, jsl],
                        in_=wsrc[:].rearrange("(t p) f -> p t f", p=128)[
                            :, :, jsl
                        ],
                    )

                def xsl(dc, csl):
                    lo = dc * R
                    return xT_all[:, lo + csl.start : lo + csl.stop]

                if "B" in phases and "C" in phases:
                    wBC = ctxX.enter_context(tc.tile_pool(name="wBC", bufs=1))
                    wq_all = wBC.tile([128, ND * DIM], F16, tag="wq")
                    wo_all = wBC.tile([128, ND * DIM], F16, tag="wo")
                    cosF_t = wBC.tile([128, R], F16, tag="cosF")
                    sinF_t = wBC.tile([128, R], F16, tag="sinF")
                    G_pool = ctxX.enter_context(
                        tc.tile_pool(name="G_pool", bufs=1)
                    )
                    G_all = G_pool.tile([128, NJ * DIM], F16, tag="G")

                def _emit_phaseB_loads():
                    if not ("B" in phases and "C" in phases):
                        return
                    nc.sync.dma_start(
                        out=wq_all[:].rearrange("p (t f) -> p t f", t=ND),
                        in_=WqT[:].rearrange("(t p) f -> p t f", p=128),
                    )
                    nc.sync.dma_start(
                        out=wo_all[:].rearrange("p (t f) -> p t f", t=ND),
                        in_=WoT[:].rearrange("(t p) f -> p t f", p=128),
                    )
                    nc.sync.dma_start(out=cosF_t[:], in_=cosF[:])
                    nc.sync.dma_start(out=sinF_t[:], in_=sinF[:])

                # ========= Phase A: k/v proj + process + kvT Grams ==========
                with ExitStack() as ctxA:
                  if "A" not in phases:
                    _emit_consts()
                    for xc in range(4):
                        _xdma(xc)
                    _emit_phaseB_loads()
                  else:
                    wA = ctxA.enter_context(tc.tile_pool(name="wA", bufs=1))
                    psA = ctxA.enter_context(
                        tc.tile_pool(name="psA", bufs=3, space="PSUM")
                    )
                    pskv = ctxA.enter_context(
                        tc.tile_pool(name="pskv", bufs=1, space="PSUM")
                    )
                    sbA = ctxA.enter_context(tc.tile_pool(name="sbA", bufs=2))
                    sb1 = ctxA.enter_context(tc.tile_pool(name="sb1", bufs=2))
                    smA = ctxA.enter_context(tc.tile_pool(name="smA", bufs=2))

                    # explicit load order on the SP dma queue: everything the
                    # phase-A pipeline needs, j/c-chunked so compute starts
                    # after ~1.5 MiB, then the phase-B weights
                    wk_all = wA.tile([128, ND * DIM], F16, tag="wk")
                    wv_all = wA.tile([128, ND * DIM], F16, tag="wv")
                    _wdma(wk_all, WkT, 0)
                    _xdma(0)
                    _wdma(wv_all, WvT, 0)
                    _emit_consts()
                    _wdma(wk_all, WkT, 1)
                    _xdma(1)
                    _wdma(wv_all, WvT, 1)
                    _xdma(2)
                    _xdma(3)
                    _emit_phaseB_loads()

                    kv_ps = pskv.tile([128, NPAIR * 128], F32, tag="kvps")
                    kv_pending = []

                    # kvT Gram: out[e,d] = sum_c v[c,e] khat[c,d]
                    # On HW start=True zeroes the whole PSUM bank, so only
                    # the first pair written to each bank may carry it.
                    def _emit_kv(item):
                        ct_, khat_, v_ = item
                        for p in range(NPAIR):
                            ps_ = slice(p * 128, (p + 1) * 128)
                            nc.tensor.matmul(
                                kv_ps[:, ps_],
                                v_[:, ps_],
                                khat_[:, ps_],
                                start=(
                                    True
                                    if sim_mode
                                    else (ct_ == 0 and p % 4 == 0)
                                ),
                                stop=(
                                    True if sim_mode else (ct_ == NC_T - 1)
                                ),
                            )

                    for ct in range(NC_T):
                        cs = slice(ct * 128, (ct + 1) * 128)
                        k_ps = psA.tile([128, DIM], F32, tag="proj_ps")
                        v_ps = psA.tile([128, DIM], F32, tag="proj_ps")
                        for half in range(2):
                            js = slice(half * 512, (half + 1) * 512)
                            for dc in range(ND):
                                nc.tensor.matmul(
                                    k_ps[:, js],
                                    xsl(dc, cs),
                                    wk_all[
                                        :, dc * DIM + js.start : dc * DIM + js.stop
                                    ],
                                    start=(dc == 0),
                                    stop=(dc == ND - 1),
                                )
                            for dc in range(ND):
                                nc.tensor.matmul(
                                    v_ps[:, js],
                                    xsl(dc, cs),
                                    wv_all[
                                        :, dc * DIM + js.start : dc * DIM + js.stop
                                    ],
                                    start=(dc == 0),
                                    stop=(dc == ND - 1),
                                )

                        # evictions: v and k on Act (cast fp16)
                        v_sb = sbA.tile([128, DIM], F16, tag="v_sb")
                        nc.scalar.activation(v_sb[:], v_ps[:], Copy)
                        k_sb = sbA.tile([128, DIM], F16, tag="k_sb")
                        nc.scalar.activation(k_sb[:], k_ps[:], Copy)
                        # squares for the l2 norm (Pool, SBUF-only)
                        sq = sbA.tile([128, DIM], F16, tag="sq")
                        nc.gpsimd.tensor_mul(sq[:], k_sb[:], k_sb[:])

                        cosb = (
                            cosC_t[:, ct * HD : (ct + 1) * HD]
                            .unsqueeze(1)
                            .broadcast_to([128, H, HD])
                        )
                        sinb4 = (
                            sinC_t[:, ct * HD : (ct + 1) * HD]
                            .rearrange("p (g two) -> p g two", two=2)
                            .unsqueeze(1)
                            .broadcast_to([128, H, HD // 2, 2])
                        )
                        k3 = k_sb[:].rearrange("p (h f) -> p h f", h=H)
                        k_sw = k_sb[:].rearrange(
                            "p (h g two) -> p h g two", h=H, two=2
                        )[:, :, :, ::-1]

                        m1 = sb1.tile([128, DIM], F16, tag="m1")
                        nc.vector.tensor_tensor(
                            m1[:].rearrange("p (h f) -> p h f", h=H), k3, cosb, MUL
                        )
                        red = smA.tile([128, H], F32, tag="red")
                        nc.vector.tensor_reduce(
                            red[:],
                            sq[:].rearrange("p (h f) -> p h f", h=H),
                            mybir.AxisListType.X,
                            ADD,
                        )
                        srt = smA.tile([128, H], F32, tag="srt")
                        nc.scalar.activation(srt[:], red[:], Sqrt)
                        rs = smA.tile([128, H], F32, tag="rs")
                        nc.vector.reciprocal(rs[:], srt[:])
                        rsm = smA.tile([128, H], F16, tag="rsm")
                        nc.vector.tensor_scalar_mul(
                            rsm[:], rs[:], maskC_t[:, ct : ct + 1]
                        )
                        m2 = sb1.tile([128, DIM], F16, tag="m2")
                        nc.vector.tensor_tensor(
                            m2[:].rearrange("p (h g two) -> p h g two", h=H, two=2),
                            k_sw,
                            sinb4,
                            MUL,
                        )
                        s = sb1.tile([128, DIM], F16, tag="s")
                        nc.gpsimd.tensor_tensor(s[:], m1[:], m2[:], ADD)
                        khat = sbA.tile([128, DIM], F16, tag="khat")
                        rsb = rsm[:].unsqueeze(2).broadcast_to([128, H, HD])
                        nc.vector.tensor_tensor(
                            khat[:].rearrange("p (h f) -> p h f", h=H),
                            s[:].rearrange("p (h f) -> p h f", h=H),
                            rsb,
                            MUL,
                        )

                        # kvT Grams are issued one iteration late (software
                        # pipelining) so PE never waits on the khat chain
                        kv_pending.append((ct, khat, v_sb))
                        if len(kv_pending) > 1:
                            _emit_kv(kv_pending.pop(0))

                    while kv_pending:
                        _emit_kv(kv_pending.pop(0))

                    # evict kvT partials (fp16) and run the collective
                    kv_sb = sbA.tile([128, NPAIR * 128], F16, tag="kv_sb")
                    nc.vector.tensor_copy(kv_sb[:], kv_ps[:])
                    nc.sync.dma_start(out=kv_in_d.ap(), in_=kv_sb[:])
                    if sim_mode or not coll:
                        nc.sync.dma_start(out=kv_out_d.ap(), in_=kv_in_d.ap())
                    else:
                        nc.gpsimd.collective_compute(
                            "AllReduce",
                            ADD,
                            replica_groups=[[0, 1, 2, 3], [4, 5, 6, 7]],
                            ins=[kv_in_d.ap().opt()],
                            outs=[kv_out_d.ap().opt()],
                        )

                # ==== Fused phase B+C: q proj/norm/rope + (attn·Wo) ===
                with ExitStack() as ctxB:
                  if "B" in phases and "C" in phases:
                    psB = ctxB.enter_context(
                        tc.tile_pool(name="psB", bufs=2, space="PSUM")
                    )
                    psN = ctxB.enter_context(
                        tc.tile_pool(name="psN", bufs=1, space="PSUM")
                    )
                    psR = ctxB.enter_context(
                        tc.tile_pool(name="psR", bufs=2, space="PSUM")
                    )
                    psO = ctxB.enter_context(
                        tc.tile_pool(name="psO", bufs=3, space="PSUM")
                    )
                    sbB = ctxB.enter_context(tc.tile_pool(name="sbB", bufs=3))
                    sbS = ctxB.enter_context(
                        tc.tile_pool(name="sbS", bufs=2 * NJ)
                    )
                    sbQ = ctxB.enter_context(
                        tc.tile_pool(name="sbQ", bufs=NQ_T)
                    )

                    def _emit_pass1(ct_):
                        """q proj + squares + norm accumulation; returns
                        (q_sbs, norms_ps)."""
                        cs = slice(ct_ * 512, (ct_ + 1) * 512)
                        norms_ps = psN.tile([16, 512], F32, tag="norms")
                        q_sbs = []
                        for jt in range(NJ):
                            jlo = jt * 128
                            q_ps = psB.tile([128, 512], F32, tag="q_ps")
                            for dc in range(ND):
                                nc.tensor.matmul(
                                    q_ps[:],
                                    wq_all[
                                        :, dc * DIM + jlo : dc * DIM + jlo + 128
                                    ],
                                    xsl(dc, cs),
                                    start=(dc == 0),
                                    stop=(dc == ND - 1),
                                )
                            q_sb = sbS.tile([128, 512], F16, tag="q_sb")
                            nc.scalar.activation(q_sb[:], q_ps[:], Copy)
                            sq = sbB.tile([128, 512], F16, tag="sqB")
                            nc.vector.tensor_mul(sq[:], q_sb[:], q_sb[:])
                            nc.tensor.matmul(
                                norms_ps[:],
                                ind16T_t[:, jt * 16 : (jt + 1) * 16],
                                sq[:],
                                start=(jt == 0),
                                stop=(jt == NJ - 1),
                            )
                            q_sbs.append(q_sb)
                        return q_sbs, norms_ps

                    def _emit_pass2(ct_, q_sbs, norms_ps):
                        """rsqrt + rope + scale; returns qh_all."""
                        cs = slice(ct_ * 512, (ct_ + 1) * 512)
                        nsrt = sbB.tile([16, 512], F32, tag="nsrt")
                        nc.scalar.activation(nsrt[:], norms_ps[:], Sqrt)
                        nrcp = sbB.tile([16, 512], F32, tag="nrcp")
                        nc.vector.reciprocal(nrcp[:], nsrt[:])
                        rs16 = sbB.tile([16, 512], F16, tag="rs16")
                        nc.vector.tensor_copy(rs16[:], nrcp[:])
                        qh_all = sbQ.tile([128, NJ * 512], F16, tag="qhall")
                        for jt in range(NJ):
                            q_sb = q_sbs[jt]
                            rot_ps = psR.tile([128, 512], F32, tag="rotrep")
                            nc.tensor.matmul(
                                rot_ps[:], P_t[:], q_sb[:], start=True, stop=True
                            )
                            rep_ps = psR.tile([128, 512], F32, tag="rotrep")
                            nc.tensor.matmul(
                                rep_ps[:],
                                ind16_t[:, jt * 128 : (jt + 1) * 128],
                                rs16[:],
                                start=True,
                                stop=True,
                            )
                            rot_sb = sbB.tile([128, 512], F16, tag="rot_sb")
                            nc.scalar.activation(rot_sb[:], rot_ps[:], Copy)
                            rep_sb = sbB.tile([128, 512], F16, tag="rep_sb")
                            nc.vector.tensor_copy(rep_sb[:], rep_ps[:])
                            t1 = sbB.tile([128, 512], F16, tag="t1")
                            nc.vector.tensor_tensor(
                                t1[:], q_sb[:], cosF_t[:, cs], MUL
                            )
                            t2 = sbB.tile([128, 512], F16, tag="t2")
                            nc.vector.tensor_tensor(
                                t2[:], rot_sb[:], sinF_t[:, cs], MUL
                            )
                            s = sbB.tile([128, 512], F16, tag="sB")
                            nc.vector.tensor_tensor(s[:], t1[:], t2[:], ADD)
                            nc.vector.tensor_tensor(
                                qh_all[:, jt * 512 : (jt + 1) * 512],
                                s[:],
                                rep_sb[:],
                                MUL,
                            )
                        return qh_all

                    def _emit_G():
                        """kvT load (blocks on collective) -> G = kvT^T@WoT."""
                        kvpool = ctxB.enter_context(
                            tc.tile_pool(name="kvpool", bufs=1)
                        )
                        kvf = kvpool.tile([128, NPAIR * 128], F16, tag="kvf")
                        nc.sync.dma_start(out=kvf[:], in_=kv_out_d.ap())
                        kvblk = kvpool.tile([128, NPAIR * 128], F16, tag="kvblk")
                        nc.vector.memset(kvblk[:], 0.0)
                        nc.vector.tensor_copy(
                            kvblk[0:64, :].rearrange(
                                "p (t f) -> p t f", t=NPAIR
                            )[:, :, 0:64],
                            kvf[0:64, :].rearrange("p (t f) -> p t f", t=NPAIR)[
                                :, :, 0:64
                            ],
                        )
                        nc.vector.tensor_copy(
                            kvblk[64:128, :].rearrange(
                                "p (t f) -> p t f", t=NPAIR
                            )[:, :, 64:128],
                            kvf[64:128, :].rearrange(
                                "p (t f) -> p t f", t=NPAIR
                            )[:, :, 64:128],
                        )
                        for hp in range(NPAIR):
                            for half in range(2):
                                fs = slice(hp * DIM + half * 512,
                                           hp * DIM + (half + 1) * 512)
                                g_ps = psO.tile([128, 512], F32, tag="o_ps")
                                nc.tensor.matmul(
                                    g_ps[:],
                                    kvblk[:, hp * 128 : (hp + 1) * 128],
                                    wo_all[:, fs],
                                    start=True,
                                    stop=True,
                                )
                                nc.scalar.activation(
                                    G_all[:, fs], g_ps[:], Copy
                                )

                    def _emit_out(ct_, qh_):
                        cs_ = slice(ct_ * 512, (ct_ + 1) * 512)
                        o_all = sbQ.tile([128, NJ * 512], F16, tag="o_all")
                        outv = out_d.rearrange("(t p) c -> p t c", p=128)
                        for et in range(NJ):
                            elo = et * 128
                            o_ps = psO.tile([128, 512], F32, tag="o_ps")
                            for jt in range(NJ):
                                nc.tensor.matmul(
                                    o_ps[:],
                                    G_all[
                                        :, jt * DIM + elo : jt * DIM + elo + 128
                                    ],
                                    qh_[:, jt * 512 : (jt + 1) * 512],
                                    start=(jt == 0),
                                    stop=(jt == NJ - 1),
                                )
                            nc.scalar.activation(
                                o_all[:, et * 512 : (et + 1) * 512], o_ps[:], Copy
                            )
                            # per-et DMA so the tail shrinks to one et
                            nc.sync.dma_start(
                                out=outv[:, et, cs_],
                                in_=o_all[:, et * 512 : (et + 1) * 512],
                            )

                    # emission order: pass1(0), pass2(0), pass1(1), pass2(1),
                    # G, out(0), out(1) — G sits behind the full q pipeline
                    # of independent PE work to hide the AllReduce latency.
                    q0, n0 = _emit_pass1(0)
                    qh0 = _emit_pass2(0, q0, n0)
                    q1, n1 = _emit_pass1(1)
                    qh1 = _emit_pass2(1, q1, n1)
                    _emit_G()
                    _emit_out(0, qh0)
                    _emit_out(1, qh1)

    nc.compile()
    return nc


_NC_CACHE = None


def _get_nc():
    global _NC_CACHE
    if _NC_CACHE is None:
        _NC_CACHE = build_nc()
    return _NC_CACHE


def _plan_rows(mask):
    """Split each batch's unmasked row indices over its 4 cores.
    Returns rows[core] = np.ndarray of original row ids (len <= R)."""
    mask = np.asarray(mask)
    rows = []
    for b in range(B):
        idx = np.flatnonzero(mask[b] != 0)
        n = len(idx)
        assert n <= 4 * R, f"unmasked rows {n} exceed capacity {4 * R}"
        szs = [n // 4 + (1 if i < n % 4 else 0) for i in range(4)]
        off = 0
        for cc in range(4):
            rows.append(idx[off : off + szs[cc]])
            off += szs[cc]
    return rows


def make_in_maps(x, mask, Wq, Wk, Wv, Wo, norm_const):
    x = np.asarray(x, np.float32)
    mask = np.asarray(mask)
    Wq = np.asarray(Wq, np.float32)
    Wk = np.asarray(Wk, np.float32)
    Wv = np.asarray(Wv, np.float32)
    Wo = np.asarray(Wo, np.float32)
    norm_const = np.asarray(norm_const, np.float32).reshape(H)

    sig = 1.0 / (1.0 + np.exp(-norm_const.astype(np.float64)))
    svec = np.float64(C) ** (-sig)  # [H]
    s_cols = np.repeat(svec, HD)  # [DIM]

    f16 = np.float16
    WkT = np.ascontiguousarray(Wk.T).astype(f16)
    WvT = np.ascontiguousarray((Wv * s_cols[:, None].astype(np.float32)).T).astype(
        f16
    )
    WqT = np.ascontiguousarray(Wq.T).astype(f16)
    WoT = np.ascontiguousarray(Wo.T).astype(f16)

    inv_freq = 1.0 / (
        ROPE_THETA ** (np.arange(0, HD, 2, dtype=np.float64) / HD)
    )  # [32]
    freq_of_j = np.repeat(inv_freq, 2)  # [64] interleaved

    ind16T = np.zeros((DIM, 16), f16)
    for jt in range(NJ):
        for kk in range(128):
            ind16T[jt * 128 + kk, 2 * jt + (kk >= 64)] = 1.0

    ind16 = np.zeros((16, DIM), f16)
    for jt in range(NJ):
        for m in range(128):
            ind16[2 * jt + (m >= 64), jt * 128 + m] = 1.0

    Pmat = np.zeros((128, 128), f16)
    for i in range(64):
        Pmat[2 * i + 1, 2 * i] = -1.0  # out[2i] = -q[2i+1]
        Pmat[2 * i, 2 * i + 1] = 1.0  # out[2i+1] = q[2i]

    rows_per_core = _plan_rows(mask)

    in_maps = []
    for core in range(N_CORES):
        b = core // (N_CORES // B)
        rows = rows_per_core[core]
        sz = len(rows)
        pos = np.zeros(R, np.float64)
        pos[:sz] = rows.astype(np.float64)

        # gathered x columns; pads filled with 1.0 (khat row is zeroed by
        # the mask so pads contribute nothing)
        xTc = np.ones((DIM, R), f16)
        xTc[:, :sz] = x[b, rows, :].T.astype(f16)

        angC = pos[:, None] * freq_of_j[None, :]  # [R, 64]
        cosCc = np.cos(angC).astype(f16)
        sinCc = np.sin(angC).astype(np.float32)
        # sign fold for the swap formulation: even j -> -sin, odd j -> +sin
        sinCc[:, 0::2] *= -1.0
        sinCc = sinCc.astype(f16)

        angF = freq_of_j[:, None] * pos[None, :]  # [64, R]
        angF2 = np.concatenate([angF, angF], axis=0)  # [128, R]
        cosFc = np.cos(angF2).astype(f16)
        sinFc = np.sin(angF2).astype(f16)

        mrow = np.zeros(R, np.float32)
        mrow[:sz] = 1.0
        maskCc = np.ascontiguousarray(mrow.reshape(NC_T, 128).T)  # [128, NC_T]

        in_maps.append(
            {
                "xT": xTc,
                "WkT": WkT,
                "WvT": WvT,
                "WqT": WqT,
                "WoT": WoT,
                "cosC": cosCc,
                "sinC": sinCc,
                "cosF": cosFc,
                "sinF": sinFc,
                "maskC": maskCc,
                "ind16T": ind16T,
                "ind16": ind16,
                "Pmat": Pmat,
            }
        )
    return in_maps


def assemble_output(results, mask):
    rows_per_core = _plan_rows(mask)
    out = np.zeros((B, C, DIM), np.float32)
    for core in range(N_CORES):
        b = core // (N_CORES // B)
        rows = rows_per_core[core]
        sz = len(rows)
        out[b, rows, :] = results[core]["out"].T[:sz].astype(np.float32)
    return out


def kernel(x, mask, Wq, Wk, Wv, Wo, norm_const):
    nc = _get_nc()
    in_maps = make_in_maps(x, mask, Wq, Wk, Wv, Wo, norm_const)
    res = run_bass_kernel_spmd(nc, in_maps, list(range(N_CORES)))
    return assemble_output(res.results, mask)
